# revision 33
# baseline (speedup 1.0000x reference)
"""Trainium2 Bass kernel for ConditionalGraphKernelNetwork (NNConv-style GNN).

Strategy (edge-parallel over 8 NeuronCores, dst-range sharded):
  - Edges are sorted by dst and sharded so core c owns all edges whose dst is
    in its contiguous 1280-node range.  The per-edge HxH kernel weights `w`
    (the dominant data, ~61MB bf16/core) are computed once on-device and
    streamed from HBM in each of the 3 message-passing layers (memory-bound
    regime).
  - Per-edge bmm  msg_e = h[src_e] @ W_e  is computed on the vector engine as
    one broadcast-multiply + one grouped reduction per 128-edge tile, with W
    stored o-major ([e, o*64+i]) so the reduction is over the innermost axis.
  - scatter-mean is a single SWDGE dma_scatter_add into a per-core local agg
    table (dst-local indices), then scaled by 1/deg per node.
  - Node features h are exchanged between layers with an AllGather; the root
    term h @ root rides the same PSUM accumulation as the scatter result via
    tensor-engine matmuls on a transposed copy of h.
"""

import os
import subprocess
import sys
import tempfile

import numpy as np

if "/opt/trn_rl_repo" not in sys.path:
    sys.path.insert(0, "/opt/trn_rl_repo")

import ml_dtypes

P = 128
BF = ml_dtypes.bfloat16

# Problem config (hardcoded per harness contract).
CFG = dict(
    N=10000, E=60000, B=4,
    NODE_IN=6, EDGE_IN=6, COND_IN=10, SCALE_IN=4,
    H=64, KW=256, OUT=1, LAYERS=3, NCORES=8,
)


def _wrap_idx(idx, emax):
    """Linear index j -> [j % 16, j // 16]; the 16-row block is replicated
    across all 128 partitions (each GPSIMD Q7 core reads its own copy)."""
    blk = np.asarray(idx, np.int16).reshape(-1, 16).T   # [16, emax//16]
    return np.tile(blk, (8, 1)).copy()


def _pad_rows(a, n, fill=0.0):
    out = np.full((n,) + a.shape[1:], fill, a.dtype)
    out[: a.shape[0]] = a
    return out


def host_prep(inputs, cfg):
    """Pure index/layout preprocessing (no float math beyond dtype casts)."""
    N, E, B = cfg["N"], cfg["E"], cfg["B"]
    H, KW = cfg["H"], cfg["KW"]
    NC = cfg["NCORES"]

    x = np.asarray(inputs["x"], np.float32)
    edge_attr = np.asarray(inputs["edge_attr"], np.float32)
    conditions = np.asarray(inputs["conditions"], np.float32)
    scale = np.asarray(inputs["scale"], np.float32)
    edge_index = np.asarray(inputs["edge_index"], np.int64)
    batch = np.asarray(inputs["batch"], np.int64)

    npad = -(-N // (NC * P)) * NC * P          # e.g. 10240
    percore = npad // NC                        # 1280
    ngroups = percore // P                      # 10

    src, dst = edge_index[0], edge_index[1]
    order = np.argsort(dst, kind="stable")
    src_s, dst_s = src[order], dst[order]

    # per-(core, 128-node-group) edge counts; common subtile structure across
    # cores so the SPMD program is identical on every core.
    ngroups_total = npad // P
    grp_counts = np.bincount(dst_s // P, minlength=ngroups_total).reshape(NC, ngroups_total // NC)
    subt = np.maximum(1, -(-grp_counts // P)).max(axis=0)     # [ngroups] per-group subtiles
    T = int(subt.sum())
    emax = T * P

    deg = np.bincount(dst, minlength=N).astype(np.float64)
    deg = np.maximum(deg, 1.0)
    recip_full = np.ones((npad,), np.float32)
    recip_full[:N] = (1.0 / deg).astype(np.float32)

    grp_off = np.concatenate([[0], np.cumsum(subt)])          # tile offset of group g

    # replicated host-side tensors
    xT = np.zeros((cfg["NODE_IN"], npad), np.float32)
    xT[:, :N] = x.T
    csT = np.concatenate([conditions, scale], axis=1).T.copy()  # [14, B]

    # weights (layout transforms only + bf16 casts)
    kW1 = np.asarray(inputs["kW1"], np.float32)       # [6+64, 128]
    kW3 = np.asarray(inputs["kW3"], np.float32)       # [256, 4096]
    kb3 = np.asarray(inputs["kb3"], np.float32)       # [4096]
    # o-major permutation: wperm[:, o*H+i] = w[:, i*H+o]
    kW3p = kW3.reshape(KW, H, H).transpose(0, 2, 1).reshape(KW, H * H)
    kb3p = kb3.reshape(H, H).T.reshape(1, H * H)

    rep = dict(
        xT=xT,
        csT=csT,
        nW1=np.asarray(inputs["nW1"], np.float32),
        nb1c=np.asarray(inputs["nb1"], np.float32).reshape(H, 1),
        nW2=np.asarray(inputs["nW2"], np.float32),
        nb2c=np.asarray(inputs["nb2"], np.float32).reshape(H, 1),
        cW1=np.asarray(inputs["cW1"], np.float32),
        cb1r=np.asarray(inputs["cb1"], np.float32).reshape(1, H),
        cW2=np.asarray(inputs["cW2"], np.float32),
        cb2c=np.asarray(inputs["cb2"], np.float32).reshape(H, 1),
        kW1a=kW1[: cfg["EDGE_IN"]].copy(),            # [6, 128]
        kW1u=kW1[cfg["EDGE_IN"]:].copy(),             # [64, 128]
        kb1r=np.asarray(inputs["kb1"], np.float32).reshape(1, KW // 2),
        kW2bf=np.asarray(inputs["kW2"], np.float32).astype(BF),   # [128, 256]
        kb2c=np.asarray(inputs["kb2"], np.float32).reshape(2, KW // 2).T.copy(),  # [128, 2]
        kW3p0=kW3p[: KW // 2].astype(BF),
        kW3p1=kW3p[KW // 2:].astype(BF),
        kb3p=kb3p.astype(BF),
        root=np.asarray(inputs["root"], np.float32),
        cbias_r=np.asarray(inputs["conv_bias"], np.float32).reshape(1, H),
        oW=np.asarray(inputs["oW"], np.float32),
        ob_r=np.asarray(inputs["ob"], np.float32).reshape(1, cfg["OUT"]),
    )

    grp_start_all = np.searchsorted(dst_s, np.arange(ngroups_total) * P)
    grp_end_all = np.searchsorted(dst_s, (np.arange(ngroups_total) + 1) * P)

    in_maps = []
    for c in range(NC):
        nsl = slice(c * percore, (c + 1) * percore)

        # slot-assign this core's edges into the common tile structure
        ea = np.zeros((emax, cfg["EDGE_IN"]), np.float32)
        oneb = np.zeros((emax, B), np.float32)
        src_pad = np.zeros((emax,), np.int16)
        dstrel = np.full((emax,), -1.0, np.float32)
        recip_e = np.zeros((emax,), np.float32)
        for g in range(ngroups):
            gg = c * ngroups + g
            s, e = int(grp_start_all[gg]), int(grp_end_all[gg])
            n = e - s
            o = int(grp_off[g]) * P
            assert n <= int(subt[g]) * P
            eidx = order[s:e]
            ea[o:o + n] = edge_attr[eidx]
            oneb[np.arange(o, o + n), batch[src_s[s:e]]] = 1.0
            src_pad[o:o + n] = src_s[s:e].astype(np.int16)
            dstrel[o:o + n] = (dst_s[s:e] - gg * P).astype(np.float32)
            recip_e[o:o + n] = recip_full[dst_s[s:e]]

        xT_own = np.ascontiguousarray(xT[:, nsl])               # [6, percore]

        m = dict(rep)
        m.update(
            eaT=np.ascontiguousarray(ea.T),                     # [6, emax]
            onehotB=np.ascontiguousarray(oneb.T),               # [4, emax] f32
            srcidx=_wrap_idx(src_pad, emax),
            dstrel=np.ascontiguousarray(dstrel.reshape(T, P).T),   # [128, T]
            recip_e=np.ascontiguousarray(recip_e.reshape(T, P).T),  # [128, T]
            xT_own=xT_own,
        )
        in_maps.append(m)

    meta = dict(T=T, emax=emax, npad=npad, percore=percore, ngroups=ngroups,
                subt=[int(v) for v in subt])
    return in_maps, meta


def build_program(cfg, meta):
    import concourse.bacc as bacc
    import concourse.mybir as mybir
    import concourse.tile as tile
    from concourse.masks import make_identity

    F32 = mybir.dt.float32
    BF16 = mybir.dt.bfloat16
    I16 = mybir.dt.int16
    I32 = mybir.dt.int32

    N, B = cfg["N"], cfg["B"]
    H, KW = cfg["H"], cfg["KW"]
    OUT = cfg["OUT"]
    NC = cfg["NCORES"]
    EIN, NIN = cfg["EDGE_IN"], cfg["NODE_IN"]
    CIN = cfg["COND_IN"] + cfg["SCALE_IN"]
    LAYERS = cfg["LAYERS"]
    T, emax = meta["T"], meta["emax"]
    npad, percore, ngroups = meta["npad"], meta["percore"], meta["ngroups"]
    nblk_global = npad // P

    nc = bacc.Bacc("TRN2", target_bir_lowering=False, debug=False, num_devices=NC)

    # ---- IO ----
    xT_i = nc.dram_tensor("xT", [NIN, npad], F32, kind="ExternalInput")
    xTo_i = nc.dram_tensor("xT_own", [NIN, percore], F32, kind="ExternalInput")
    csT_i = nc.dram_tensor("csT", [CIN, B], F32, kind="ExternalInput")
    eaT_i = nc.dram_tensor("eaT", [EIN, emax], F32, kind="ExternalInput")
    oneb_i = nc.dram_tensor("onehotB", [B, emax], F32, kind="ExternalInput")
    srci_i = nc.dram_tensor("srcidx", [P, emax // 16], I16, kind="ExternalInput")
    dstr_i = nc.dram_tensor("dstrel", [P, T], F32, kind="ExternalInput")
    rcpe_i = nc.dram_tensor("recip_e", [P, T], F32, kind="ExternalInput")

    wt = {}
    for name, shape, dt in [
        ("nW1", [NIN, H], F32), ("nb1c", [H, 1], F32),
        ("nW2", [H, H], F32), ("nb2c", [H, 1], F32),
        ("cW1", [CIN, H], F32), ("cb1r", [1, H], F32),
        ("cW2", [H, H], F32), ("cb2c", [H, 1], F32),
        ("kW1a", [EIN, KW // 2], F32), ("kW1u", [H, KW // 2], F32),
        ("kb1r", [1, KW // 2], F32),
        ("kW2bf", [KW // 2, KW], BF16), ("kb2c", [KW // 2, 2], F32),
        ("kW3p0", [KW // 2, H * H], BF16), ("kW3p1", [KW // 2, H * H], BF16),
        ("kb3p", [1, H * H], BF16),
        ("root", [H, H], F32), ("cbias_r", [1, H], F32),
        ("oW", [H, OUT], F32), ("ob_r", [1, OUT], F32),
    ]:
        wt[name] = nc.dram_tensor(name, shape, dt, kind="ExternalInput")

    out_o = nc.dram_tensor("out_o", [percore, OUT], F32, kind="ExternalOutput")
    u_o = nc.dram_tensor("u_o", [B, H], F32, kind="ExternalOutput")

    # internal DRAM
    w_dram = nc.dram_tensor("w_dram", [emax, H * H], BF16)
    h0_dram = nc.dram_tensor("h0_dram", [npad, H], F32)
    subt = meta["subt"]

    WCHUNK = 512
    NCH = (H * H) // WCHUNK

    with tile.TileContext(nc, num_cores=NC) as tc:
        with tc.tile_pool(name="persist", bufs=1) as pp, \
             tc.tile_pool(name="dram", bufs=1, space="DRAM") as dp:

            # ---------- persistent sbuf ----------
            ident = pp.tile([P, P], F32, tag="ident")
            make_identity(nc, ident[:])
            ones_r = pp.tile([1, P], F32, tag="ones_r")
            nc.vector.memset(ones_r[:], 1.0)
            ones_bf = pp.tile([1, P], BF16, tag="ones_bf")
            nc.vector.memset(ones_bf[:], 1.0)

            wsb = {}
            for name in ["nW1", "nb1c", "nW2", "nb2c", "cW1", "cb1r", "cW2",
                         "cb2c", "kW1a", "kW1u", "kb1r", "kW2bf", "kb2c",
                         "kW3p0", "kW3p1", "kb3p", "root", "cbias_r", "oW", "ob_r"]:
                t = pp.tile(list(wt[name].shape), wt[name].dtype, tag=f"w_{name}")
                nc.sync.dma_start(t[:], wt[name][:])
                wsb[name] = t

            srci_t = pp.tile([P, emax // 16], I16, tag="srci")
            nc.sync.dma_start(srci_t[:], srci_i[:])
            dstr_t = pp.tile([P, T], F32, tag="dstr")
            nc.sync.dma_start(dstr_t[:], dstr_i[:])
            rcpe_t = pp.tile([P, T], F32, tag="rcpe")
            nc.sync.dma_start(rcpe_t[:], rcpe_i[:])

            iota_i32 = pp.tile([P, P], I32, tag="iota_i")
            nc.gpsimd.iota(iota_i32[:], pattern=[[1, P]], base=0, channel_multiplier=0)
            iota_f = pp.tile([P, P], F32, tag="iota_f")
            nc.vector.tensor_copy(iota_f[:], iota_i32[:])

            ubias = pp.tile([B, KW // 2], F32, tag="ubias")
            hT_a = pp.tile([H, percore], F32, tag="hT_a")
            hT_b = pp.tile([H, percore], F32, tag="hT_b")
            out_sb = pp.tile([P, ngroups], F32, tag="out_sb")

            # collective bounce buffers
            cc_in = [dp.tile([percore, H], F32, tag=f"ccin{l}", name=f"ccin{l}")
                     for l in range(LAYERS - 1)]
            cc_out = [dp.tile([npad, H], F32, tag=f"ccout{l}", name=f"ccout{l}")
                      for l in range(LAYERS - 1)]

            # ---------- phase 0: encoders ----------
            with tc.tile_pool(name="ph0big", bufs=1) as p0big, \
                 tc.tile_pool(name="ph0", bufs=2) as p0, \
                 tc.tile_pool(name="ph0ps", bufs=2, space="PSUM") as ps0, \
                 tc.tile_pool(name="ph0ps2", bufs=2, space="PSUM") as ps0b:

                # condition encoder -> uT [H, B], ubias [B, 128]
                csT_t = p0.tile([CIN, B], F32, tag="csT")
                nc.sync.dma_start(csT_t[:], csT_i[:])
                u1_ps = ps0.tile([B, H], F32, tag="mm")
                nc.tensor.matmul(u1_ps[:], lhsT=csT_t[:], rhs=wsb["cW1"][:], start=True, stop=False)
                nc.tensor.matmul(u1_ps[:], lhsT=ones_r[:, :B], rhs=wsb["cb1r"][:], start=False, stop=True)
                cu1 = p0.tile([B, H], F32, tag="cu1")
                nc.scalar.activation(cu1[:], u1_ps[:], mybir.ActivationFunctionType.Relu)

                c1t_ps = ps0b.tile([H, B], F32, tag="tp")
                nc.tensor.transpose(c1t_ps[:], in_=cu1[:], identity=ident[:B, :B])
                cu1T = p0.tile([H, B], F32, tag="cu1T")
                nc.vector.tensor_copy(cu1T[:], c1t_ps[:])

                uT_ps = ps0.tile([H, B], F32, tag="mm")
                nc.tensor.matmul(uT_ps[:], lhsT=wsb["cW2"][:], rhs=cu1T[:], start=True, stop=True)
                uT = p0.tile([H, B], F32, tag="uT")
                nc.scalar.activation(uT[:], uT_ps[:], mybir.ActivationFunctionType.Identity,
                                     bias=wsb["cb2c"][:, 0:1])

                ub_ps = ps0.tile([B, KW // 2], F32, tag="mm")
                nc.tensor.matmul(ub_ps[:], lhsT=uT[:], rhs=wsb["kW1u"][:], start=True, stop=False)
                nc.tensor.matmul(ub_ps[:], lhsT=ones_r[:, :B], rhs=wsb["kb1r"][:], start=False, stop=True)
                nc.scalar.activation(ubias[:], ub_ps[:], mybir.ActivationFunctionType.Copy)

                u_ps = ps0b.tile([B, H], F32, tag="tp")
                nc.tensor.transpose(u_ps[:], in_=uT[:], identity=ident[:H, :H])
                u_sb = p0.tile([B, H], F32, tag="u_sb")
                nc.vector.tensor_copy(u_sb[:], u_ps[:])
                nc.sync.dma_start(u_o[:], u_sb[:])

                # node encoder: full table (replicated) -> h0_dram
                xT_t = p0big.tile([NIN, npad], F32, tag="xT")
                nc.sync.dma_start(xT_t[:], xT_i[:])
                for gg in range(nblk_global):
                    sl = slice(gg * P, (gg + 1) * P)
                    t1_ps = ps0.tile([H, P], F32, tag="mm")
                    nc.tensor.matmul(t1_ps[:], lhsT=wsb["nW1"][:], rhs=xT_t[:, sl], start=True, stop=True)
                    t1 = p0.tile([H, P], F32, tag="t1")
                    nc.scalar.activation(t1[:], t1_ps[:], mybir.ActivationFunctionType.Relu,
                                         bias=wsb["nb1c"][:, 0:1])
                    h0_ps = ps0.tile([H, P], F32, tag="mm")
                    nc.tensor.matmul(h0_ps[:], lhsT=wsb["nW2"][:], rhs=t1[:], start=True, stop=True)
                    h0T_g = p0.tile([H, P], F32, tag="h0T")
                    nc.scalar.activation(h0T_g[:], h0_ps[:], mybir.ActivationFunctionType.Identity,
                                         bias=wsb["nb2c"][:, 0:1])
                    tp_ps = ps0b.tile([P, H], F32, tag="tp")
                    nc.tensor.transpose(tp_ps[:], in_=h0T_g[:], identity=ident[:H, :H])
                    h0n = p0.tile([P, H], F32, tag="h0n")
                    nc.vector.tensor_copy(h0n[:], tp_ps[:])
                    nc.sync.dma_start(h0_dram[sl, :], h0n[:])

                # own-slice transposed h0 -> hT_a
                xTo_t = p0big.tile([NIN, percore], F32, tag="xTo")
                nc.sync.dma_start(xTo_t[:], xTo_i[:])
                for g in range(ngroups):
                    sl = slice(g * P, (g + 1) * P)
                    t1_ps = ps0.tile([H, P], F32, tag="mm")
                    nc.tensor.matmul(t1_ps[:], lhsT=wsb["nW1"][:], rhs=xTo_t[:, sl], start=True, stop=True)
                    t1 = p0.tile([H, P], F32, tag="t1")
                    nc.scalar.activation(t1[:], t1_ps[:], mybir.ActivationFunctionType.Relu,
                                         bias=wsb["nb1c"][:, 0:1])
                    h0_ps = ps0.tile([H, P], F32, tag="mm")
                    nc.tensor.matmul(h0_ps[:], lhsT=wsb["nW2"][:], rhs=t1[:], start=True, stop=True)
                    nc.scalar.activation(hT_a[:, sl], h0_ps[:], mybir.ActivationFunctionType.Identity,
                                         bias=wsb["nb2c"][:, 0:1])

            # ---------- phase 1: kernel network -> w_dram ----------
            with tc.tile_pool(name="ph1big", bufs=1) as p1big, \
                 tc.tile_pool(name="ph1", bufs=2) as p1, \
                 tc.tile_pool(name="ph1w", bufs=2) as p1w, \
                 tc.tile_pool(name="ph1ps", bufs=3, space="PSUM") as ps1, \
                 tc.tile_pool(name="ph1psw", bufs=3, space="PSUM") as ps1w:

                eaT_t = p1big.tile([EIN, emax], F32, tag="eaT")
                nc.sync.dma_start(eaT_t[:], eaT_i[:])
                onebf = p1big.tile([B, emax], F32, tag="onebf")
                nc.sync.dma_start(onebf[:], oneb_i[:])

                for t in range(T):
                    sl = slice(t * P, (t + 1) * P)
                    k1_ps = ps1.tile([KW // 2, P], F32, tag="k")
                    nc.tensor.matmul(k1_ps[:], lhsT=wsb["kW1a"][:], rhs=eaT_t[:, sl],
                                     start=True, stop=False)
                    nc.tensor.matmul(k1_ps[:], lhsT=ubias[:], rhs=onebf[:, sl],
                                     start=False, stop=True)
                    k1T = p1.tile([KW // 2, P], BF16, tag="k1T")
                    nc.scalar.activation(k1T[:], k1_ps[:], mybir.ActivationFunctionType.Relu)

                    k2T = []
                    for half in range(2):
                        k2_ps = ps1.tile([KW // 2, P], F32, tag="k")
                        nc.tensor.matmul(k2_ps[:], lhsT=wsb["kW2bf"][:, half * 128:(half + 1) * 128],
                                         rhs=k1T[:], start=True, stop=True)
                        k2h = p1.tile([KW // 2, P], BF16, tag=f"k2T{half}")
                        nc.scalar.activation(k2h[:], k2_ps[:], mybir.ActivationFunctionType.Relu,
                                             bias=wsb["kb2c"][:, half:half + 1])
                        k2T.append(k2h)

                    w_sb = p1w.tile([P, H * H], BF16, tag="w_sb")
                    for chunk in range(NCH):
                        csl = slice(chunk * WCHUNK, (chunk + 1) * WCHUNK)
                        w_ps = ps1w.tile([P, WCHUNK], F32, tag="w")
                        nc.tensor.matmul(w_ps[:], lhsT=k2T[0][:], rhs=wsb["kW3p0"][:, csl],
                                         start=True, stop=False)
                        nc.tensor.matmul(w_ps[:], lhsT=k2T[1][:], rhs=wsb["kW3p1"][:, csl],
                                         start=False, stop=False)
                        nc.tensor.matmul(w_ps[:], lhsT=ones_bf[:], rhs=wsb["kb3p"][:, csl],
                                         start=False, stop=True)
                        if chunk % 2 == 0:
                            nc.scalar.activation(w_sb[:, csl], w_ps[:], mybir.ActivationFunctionType.Copy)
                        else:
                            nc.vector.tensor_copy(w_sb[:, csl], w_ps[:])
                    nc.sync.dma_start(w_dram[sl, :], w_sb[:])

            # ---------- layers ----------
            with tc.tile_pool(name="lay", bufs=1) as pl, \
                 tc.tile_pool(name="layw", bufs=3) as plw, \
                 tc.tile_pool(name="laytmp", bufs=2) as plt, \
                 tc.tile_pool(name="layps", bufs=3, space="PSUM") as psl, \
                 tc.tile_pool(name="laytp", bufs=3, space="PSUM") as pst:

                hT_cur, hT_next = hT_a, hT_b
                for l in range(LAYERS):
                    table = h0_dram if l == 0 else cc_out[l - 1]

                    hsrc = pl.tile([P, T * H], F32, tag="hsrc")
                    GCH = 8  # tiles per gather chunk (1024 descriptors)
                    for c0 in range(0, T, GCH):
                        tc_n = min(GCH, T - c0)
                        nc.gpsimd.dma_gather(
                            out_ap=hsrc[:, c0 * H:(c0 + tc_n) * H].rearrange(
                                "p (t e) -> p t e", t=tc_n),
                            in_ap=table[:, :],
                            idxs_ap=srci_t[:, c0 * 8:(c0 + tc_n) * 8],
                            num_idxs=tc_n * P,
                            num_idxs_reg=tc_n * P,
                            elem_size=H,
                        )
                    hsrc_bf = pl.tile([P, T * H], BF16, tag="hsrc_bf")
                    nc.vector.tensor_copy(hsrc_bf[:], hsrc[:])

                    tile_idx = 0
                    for g in range(ngroups):
                        gsl = slice(g * P, (g + 1) * P)
                        r_ps = psl.tile([P, H], F32, tag="g")
                        nc.tensor.matmul(r_ps[:], lhsT=hT_cur[:, gsl], rhs=wsb["root"][:],
                                         start=True, stop=False)
                        for _s in range(subt[g]):
                            t = tile_idx
                            tile_idx += 1
                            w_t = plw.tile([P, H * H], BF16, tag="w_t")
                            nc.sync.dma_start(w_t[:], w_dram[t * P:(t + 1) * P, :])
                            tmp = plt.tile([P, H * H], BF16, tag="tmp")
                            nc.vector.tensor_tensor(
                                out=tmp[:].rearrange("p (o i) -> p o i", o=H),
                                in0=w_t[:].rearrange("p (o i) -> p o i", o=H),
                                in1=hsrc_bf[:, t * H:(t + 1) * H][:, None, :].to_broadcast([P, H, H]),
                                op=mybir.AluOpType.mult,
                            )
                            msg_t = plt.tile([P, H], F32, tag="msg_t")
                            nc.vector.tensor_reduce(
                                out=msg_t[:],
                                in_=tmp[:].rearrange("p (o i) -> p o i", o=H),
                                axis=mybir.AxisListType.X,
                                op=mybir.AluOpType.add,
                            )
                            s_t = plt.tile([P, P], F32, tag="s_t")
                            nc.vector.tensor_scalar(
                                out=s_t[:], in0=iota_f[:],
                                scalar1=dstr_t[:, t:t + 1], scalar2=rcpe_t[:, t:t + 1],
                                op0=mybir.AluOpType.is_equal, op1=mybir.AluOpType.mult)
                            nc.tensor.matmul(r_ps[:], lhsT=s_t[:], rhs=msg_t[:],
                                             start=False, stop=False)
                        nc.tensor.matmul(r_ps[:], lhsT=ones_r[:], rhs=wsb["cbias_r"][:],
                                         start=False, stop=True)
                        h_new = plt.tile([P, H], F32, tag="h_new")
                        nc.scalar.activation(h_new[:], r_ps[:], mybir.ActivationFunctionType.Relu)

                        tp_ps = pst.tile([H, P], F32, tag="tp")
                        nc.tensor.transpose(tp_ps[:], in_=h_new[:], identity=ident[:])
                        if l < LAYERS - 1:
                            nc.sync.dma_start(cc_in[l][gsl, :], h_new[:])
                            nc.vector.tensor_copy(hT_next[:, gsl], tp_ps[:])
                        else:
                            h3T_g = plt.tile([H, P], F32, tag="h3T")
                            nc.vector.tensor_copy(h3T_g[:], tp_ps[:])
                            o_ps = psl.tile([P, OUT], F32, tag="g")
                            nc.tensor.matmul(o_ps[:], lhsT=h3T_g[:], rhs=wsb["oW"][:],
                                             start=True, stop=False)
                            nc.tensor.matmul(o_ps[:], lhsT=ones_r[:], rhs=wsb["ob_r"][:],
                                             start=False, stop=True)
                            nc.scalar.activation(out_sb[:, g:g + 1], o_ps[:],
                                                 mybir.ActivationFunctionType.Copy)

                    if l < LAYERS - 1:
                        nc.gpsimd.collective_compute(
                            "AllGather",
                            mybir.AluOpType.bypass,
                            replica_groups=[list(range(NC))],
                            ins=[cc_in[l][:].opt()],
                            outs=[cc_out[l][:].opt()],
                        )
                        hT_cur, hT_next = hT_next, hT_cur

                nc.sync.dma_start(
                    out_o[:].rearrange("(g p) one -> p g one", p=P),
                    out_sb[:, :, None])

    nc.compile()
    return nc


def run_device(inputs, cfg=None, trace=False):
    """Build + execute on the 8 NeuronCores; needs jax to see the axon devices."""
    from concourse import bass_utils

    cfg = cfg or CFG
    in_maps, meta = host_prep(inputs, cfg)
    nc = build_program(cfg, meta)
    res = bass_utils.run_bass_kernel_spmd(
        nc, in_maps, core_ids=list(range(cfg["NCORES"])), trace=trace)
    out = np.concatenate([res.results[c]["out_o"] for c in range(cfg["NCORES"])], axis=0)
    out = out[: cfg["N"]]
    u = res.results[0]["u_o"]
    return (out.astype(np.float32), u.astype(np.float32)), res


def _neuron_devices_visible():
    try:
        import jax

        return len(jax.devices()) >= CFG["NCORES"]
    except Exception:
        return False


def kernel(**inputs):
    if _neuron_devices_visible():
        (out, u), _ = run_device(inputs)
        return out, u

    # jax in this process cannot see the NeuronCores (e.g. JAX_PLATFORMS=cpu
    # was set for the host-side reference) — re-exec in a clean subprocess.
    with tempfile.TemporaryDirectory() as td:
        np.savez(os.path.join(td, "in.npz"),
                 **{k: np.asarray(v) for k, v in inputs.items()})
        env = dict(os.environ)
        for k in ["JAX_PLATFORMS", "JAX_PLATFORM_NAME"]:
            if "cpu" in env.get(k, "").lower():
                env.pop(k)
        subprocess.run(
            [sys.executable, os.path.abspath(__file__), "--runner", td],
            check=True, env=env,
        )
        with np.load(os.path.join(td, "out.npz")) as z:
            return z["out"], z["u"]


def _runner_main(td):
    with np.load(os.path.join(td, "in.npz")) as z:
        inputs = {k: z[k] for k in z.files}
    (out, u), _ = run_device(inputs)
    np.savez(os.path.join(td, "out.npz"), out=out, u=u)


if __name__ == "__main__" and len(sys.argv) == 3 and sys.argv[1] == "--runner":
    _runner_main(sys.argv[2])


# revision 34
# speedup vs baseline: 1.1808x; 1.1808x over previous
"""Trainium2 Bass kernel for ConditionalGraphKernelNetwork (NNConv-style GNN).

Strategy (edge-parallel over 8 NeuronCores, dst-range sharded):
  - Edges are sorted by dst and sharded so core c owns all edges whose dst is
    in its contiguous 1280-node range.  The per-edge HxH kernel weights `w`
    (the dominant data, ~61MB bf16/core) are computed once on-device and
    streamed from HBM in each of the 3 message-passing layers (memory-bound
    regime).
  - Per-edge bmm  msg_e = h[src_e] @ W_e  is computed on the vector engine as
    one broadcast-multiply + one grouped reduction per 128-edge tile, with W
    stored o-major ([e, o*64+i]) so the reduction is over the innermost axis.
  - scatter-mean is a single SWDGE dma_scatter_add into a per-core local agg
    table (dst-local indices), then scaled by 1/deg per node.
  - Node features h are exchanged between layers with an AllGather; the root
    term h @ root rides the same PSUM accumulation as the scatter result via
    tensor-engine matmuls on a transposed copy of h.
"""

import os
import subprocess
import sys
import tempfile

import numpy as np

if "/opt/trn_rl_repo" not in sys.path:
    sys.path.insert(0, "/opt/trn_rl_repo")

import ml_dtypes

P = 128
BF = ml_dtypes.bfloat16

# Problem config (hardcoded per harness contract).
CFG = dict(
    N=10000, E=60000, B=4,
    NODE_IN=6, EDGE_IN=6, COND_IN=10, SCALE_IN=4,
    H=64, KW=256, OUT=1, LAYERS=3, NCORES=8,
)


def _wrap_idx(idx, emax):
    """Linear index j -> [j % 16, j // 16]; the 16-row block is replicated
    across all 128 partitions (each GPSIMD Q7 core reads its own copy)."""
    blk = np.asarray(idx, np.int16).reshape(-1, 16).T   # [16, emax//16]
    return np.tile(blk, (8, 1)).copy()


def _pad_rows(a, n, fill=0.0):
    out = np.full((n,) + a.shape[1:], fill, a.dtype)
    out[: a.shape[0]] = a
    return out


def host_prep(inputs, cfg):
    """Pure index/layout preprocessing (no float math beyond dtype casts)."""
    N, E, B = cfg["N"], cfg["E"], cfg["B"]
    H, KW = cfg["H"], cfg["KW"]
    NC = cfg["NCORES"]

    x = np.asarray(inputs["x"], np.float32)
    edge_attr = np.asarray(inputs["edge_attr"], np.float32)
    conditions = np.asarray(inputs["conditions"], np.float32)
    scale = np.asarray(inputs["scale"], np.float32)
    edge_index = np.asarray(inputs["edge_index"], np.int64)
    batch = np.asarray(inputs["batch"], np.int64)

    npad = -(-N // (NC * P)) * NC * P          # e.g. 10240
    percore = npad // NC                        # 1280
    ngroups = percore // P                      # 10

    src, dst = edge_index[0], edge_index[1]
    order = np.argsort(dst, kind="stable")
    src_s, dst_s = src[order], dst[order]

    # per-(core, 128-node-group) edge counts; common subtile structure across
    # cores so the SPMD program is identical on every core.
    ngroups_total = npad // P
    grp_counts = np.bincount(dst_s // P, minlength=ngroups_total).reshape(NC, ngroups_total // NC)
    subt = np.maximum(1, -(-grp_counts // P)).max(axis=0)     # [ngroups] per-group subtiles
    T = int(subt.sum())
    emax = T * P

    deg = np.bincount(dst, minlength=N).astype(np.float64)
    deg = np.maximum(deg, 1.0)
    recip_full = np.ones((npad,), np.float32)
    recip_full[:N] = (1.0 / deg).astype(np.float32)

    grp_off = np.concatenate([[0], np.cumsum(subt)])          # tile offset of group g

    # replicated host-side tensors
    xT = np.zeros((cfg["NODE_IN"], npad), np.float32)
    xT[:, :N] = x.T
    csT = np.concatenate([conditions, scale], axis=1).T.copy()  # [14, B]

    # weights (layout transforms only + bf16 casts)
    kW1 = np.asarray(inputs["kW1"], np.float32)       # [6+64, 128]
    kW3 = np.asarray(inputs["kW3"], np.float32)       # [256, 4096]
    kb3 = np.asarray(inputs["kb3"], np.float32)       # [4096]
    # o-major permutation: wperm[:, o*H+i] = w[:, i*H+o]
    kW3p = kW3.reshape(KW, H, H).transpose(0, 2, 1).reshape(KW, H * H)
    kb3p = kb3.reshape(H, H).T.reshape(1, H * H)

    rep = dict(
        xT=xT,
        csT=csT,
        nW1=np.asarray(inputs["nW1"], np.float32),
        nb1c=np.asarray(inputs["nb1"], np.float32).reshape(H, 1),
        nW2=np.asarray(inputs["nW2"], np.float32),
        nb2c=np.asarray(inputs["nb2"], np.float32).reshape(H, 1),
        cW1=np.asarray(inputs["cW1"], np.float32),
        cb1r=np.asarray(inputs["cb1"], np.float32).reshape(1, H),
        cW2=np.asarray(inputs["cW2"], np.float32),
        cb2c=np.asarray(inputs["cb2"], np.float32).reshape(H, 1),
        kW1a=kW1[: cfg["EDGE_IN"]].copy(),            # [6, 128]
        kW1u=kW1[cfg["EDGE_IN"]:].copy(),             # [64, 128]
        kb1r=np.asarray(inputs["kb1"], np.float32).reshape(1, KW // 2),
        kW2bf=np.asarray(inputs["kW2"], np.float32).astype(BF),   # [128, 256]
        kb2c=np.asarray(inputs["kb2"], np.float32).reshape(2, KW // 2).T.copy(),  # [128, 2]
        kW3p0=kW3p[: KW // 2].astype(BF),
        kW3p1=kW3p[KW // 2:].astype(BF),
        kb3p=kb3p.astype(BF),
        root=np.asarray(inputs["root"], np.float32),
        cbias_r=np.asarray(inputs["conv_bias"], np.float32).reshape(1, H),
        oW=np.asarray(inputs["oW"], np.float32),
        ob_r=np.asarray(inputs["ob"], np.float32).reshape(1, cfg["OUT"]),
    )

    grp_start_all = np.searchsorted(dst_s, np.arange(ngroups_total) * P)
    grp_end_all = np.searchsorted(dst_s, (np.arange(ngroups_total) + 1) * P)

    in_maps = []
    for c in range(NC):
        nsl = slice(c * percore, (c + 1) * percore)

        # slot-assign this core's edges into the common tile structure
        ea = np.zeros((emax, cfg["EDGE_IN"]), np.float32)
        oneb = np.zeros((emax, B), np.float32)
        src_pad = np.zeros((emax,), np.int16)
        dstrel = np.full((emax,), -1.0, np.float32)
        recip_e = np.zeros((emax,), np.float32)
        for g in range(ngroups):
            gg = c * ngroups + g
            s, e = int(grp_start_all[gg]), int(grp_end_all[gg])
            n = e - s
            o = int(grp_off[g]) * P
            assert n <= int(subt[g]) * P
            eidx = order[s:e]
            ea[o:o + n] = edge_attr[eidx]
            oneb[np.arange(o, o + n), batch[src_s[s:e]]] = 1.0
            src_pad[o:o + n] = src_s[s:e].astype(np.int16)
            dstrel[o:o + n] = (dst_s[s:e] - gg * P).astype(np.float32)
            recip_e[o:o + n] = recip_full[dst_s[s:e]]

        xT_own = np.ascontiguousarray(xT[:, nsl])               # [6, percore]

        m = dict(rep)
        m.update(
            eaT=np.ascontiguousarray(ea.T),                     # [6, emax]
            onehotB=np.ascontiguousarray(oneb.T),               # [4, emax] f32
            srcidx=_wrap_idx(src_pad, emax),
            dstrel=np.ascontiguousarray(dstrel.reshape(T, P).T),   # [128, T]
            recip_e=np.ascontiguousarray(recip_e.reshape(T, P).T),  # [128, T]
            xT_own=xT_own,
        )
        in_maps.append(m)

    meta = dict(T=T, emax=emax, npad=npad, percore=percore, ngroups=ngroups,
                subt=[int(v) for v in subt])
    return in_maps, meta


def build_program(cfg, meta):
    import concourse.bacc as bacc
    import concourse.mybir as mybir
    import concourse.tile as tile
    from concourse.masks import make_identity

    F32 = mybir.dt.float32
    BF16 = mybir.dt.bfloat16
    I16 = mybir.dt.int16
    I32 = mybir.dt.int32

    N, B = cfg["N"], cfg["B"]
    H, KW = cfg["H"], cfg["KW"]
    OUT = cfg["OUT"]
    NC = cfg["NCORES"]
    EIN, NIN = cfg["EDGE_IN"], cfg["NODE_IN"]
    CIN = cfg["COND_IN"] + cfg["SCALE_IN"]
    LAYERS = cfg["LAYERS"]
    T, emax = meta["T"], meta["emax"]
    npad, percore, ngroups = meta["npad"], meta["percore"], meta["ngroups"]
    nblk_global = npad // P

    nc = bacc.Bacc("TRN2", target_bir_lowering=False, debug=False, num_devices=NC)

    # ---- IO ----
    xT_i = nc.dram_tensor("xT", [NIN, npad], F32, kind="ExternalInput")
    xTo_i = nc.dram_tensor("xT_own", [NIN, percore], F32, kind="ExternalInput")
    csT_i = nc.dram_tensor("csT", [CIN, B], F32, kind="ExternalInput")
    eaT_i = nc.dram_tensor("eaT", [EIN, emax], F32, kind="ExternalInput")
    oneb_i = nc.dram_tensor("onehotB", [B, emax], F32, kind="ExternalInput")
    srci_i = nc.dram_tensor("srcidx", [P, emax // 16], I16, kind="ExternalInput")
    dstr_i = nc.dram_tensor("dstrel", [P, T], F32, kind="ExternalInput")
    rcpe_i = nc.dram_tensor("recip_e", [P, T], F32, kind="ExternalInput")

    wt = {}
    for name, shape, dt in [
        ("nW1", [NIN, H], F32), ("nb1c", [H, 1], F32),
        ("nW2", [H, H], F32), ("nb2c", [H, 1], F32),
        ("cW1", [CIN, H], F32), ("cb1r", [1, H], F32),
        ("cW2", [H, H], F32), ("cb2c", [H, 1], F32),
        ("kW1a", [EIN, KW // 2], F32), ("kW1u", [H, KW // 2], F32),
        ("kb1r", [1, KW // 2], F32),
        ("kW2bf", [KW // 2, KW], BF16), ("kb2c", [KW // 2, 2], F32),
        ("kW3p0", [KW // 2, H * H], BF16), ("kW3p1", [KW // 2, H * H], BF16),
        ("kb3p", [1, H * H], BF16),
        ("root", [H, H], F32), ("cbias_r", [1, H], F32),
        ("oW", [H, OUT], F32), ("ob_r", [1, OUT], F32),
    ]:
        wt[name] = nc.dram_tensor(name, shape, dt, kind="ExternalInput")

    out_o = nc.dram_tensor("out_o", [percore, OUT], F32, kind="ExternalOutput")
    u_o = nc.dram_tensor("u_o", [B, H], F32, kind="ExternalOutput")

    # internal DRAM
    w_dram = nc.dram_tensor("w_dram", [emax, H * H], BF16)
    h0_dram = nc.dram_tensor("h0_dram", [npad, H], F32)
    subt = meta["subt"]

    WCHUNK = 512
    NCH = (H * H) // WCHUNK

    with tile.TileContext(nc, num_cores=NC) as tc:
        with tc.tile_pool(name="persist", bufs=1) as pp, \
             tc.tile_pool(name="dram", bufs=1, space="DRAM") as dp:

            # ---------- persistent sbuf ----------
            ident = pp.tile([P, P], F32, tag="ident")
            make_identity(nc, ident[:])
            ones_r = pp.tile([1, P], F32, tag="ones_r")
            nc.vector.memset(ones_r[:], 1.0)
            ones_bf = pp.tile([1, P], BF16, tag="ones_bf")
            nc.vector.memset(ones_bf[:], 1.0)

            wsb = {}
            for name in ["nW1", "nb1c", "nW2", "nb2c", "cW1", "cb1r", "cW2",
                         "cb2c", "kW1a", "kW1u", "kb1r", "kW2bf", "kb2c",
                         "kW3p0", "kW3p1", "kb3p", "root", "cbias_r", "oW", "ob_r"]:
                t = pp.tile(list(wt[name].shape), wt[name].dtype, tag=f"w_{name}")
                nc.sync.dma_start(t[:], wt[name][:])
                wsb[name] = t

            srci_t = pp.tile([P, emax // 16], I16, tag="srci")
            nc.sync.dma_start(srci_t[:], srci_i[:])
            dstr_t = pp.tile([P, T], F32, tag="dstr")
            nc.sync.dma_start(dstr_t[:], dstr_i[:])
            rcpe_t = pp.tile([P, T], F32, tag="rcpe")
            nc.sync.dma_start(rcpe_t[:], rcpe_i[:])

            iota_i32 = pp.tile([P, P], I32, tag="iota_i")
            nc.gpsimd.iota(iota_i32[:], pattern=[[1, P]], base=0, channel_multiplier=0)
            iota_f = pp.tile([P, P], F32, tag="iota_f")
            nc.vector.tensor_copy(iota_f[:], iota_i32[:])

            ubias = pp.tile([B, KW // 2], F32, tag="ubias")
            hT_a = pp.tile([H, percore], F32, tag="hT_a")
            hT_b = pp.tile([H, percore], F32, tag="hT_b")
            out_sb = pp.tile([P, ngroups], F32, tag="out_sb")

            # collective bounce buffers
            cc_in = [dp.tile([percore, H], F32, tag=f"ccin{l}", name=f"ccin{l}")
                     for l in range(LAYERS - 1)]
            cc_out = [dp.tile([npad, H], F32, tag=f"ccout{l}", name=f"ccout{l}")
                      for l in range(LAYERS - 1)]

            # ---------- phase 0: encoders ----------
            with tc.tile_pool(name="ph0big", bufs=1) as p0big, \
                 tc.tile_pool(name="ph0", bufs=2) as p0, \
                 tc.tile_pool(name="ph0ps", bufs=2, space="PSUM") as ps0, \
                 tc.tile_pool(name="ph0ps2", bufs=2, space="PSUM") as ps0b:

                # condition encoder -> uT [H, B], ubias [B, 128]
                csT_t = p0.tile([CIN, B], F32, tag="csT")
                nc.sync.dma_start(csT_t[:], csT_i[:])
                u1_ps = ps0.tile([B, H], F32, tag="mm")
                nc.tensor.matmul(u1_ps[:], lhsT=csT_t[:], rhs=wsb["cW1"][:], start=True, stop=False)
                nc.tensor.matmul(u1_ps[:], lhsT=ones_r[:, :B], rhs=wsb["cb1r"][:], start=False, stop=True)
                cu1 = p0.tile([B, H], F32, tag="cu1")
                nc.scalar.activation(cu1[:], u1_ps[:], mybir.ActivationFunctionType.Relu)

                c1t_ps = ps0b.tile([H, B], F32, tag="tp")
                nc.tensor.transpose(c1t_ps[:], in_=cu1[:], identity=ident[:B, :B])
                cu1T = p0.tile([H, B], F32, tag="cu1T")
                nc.vector.tensor_copy(cu1T[:], c1t_ps[:])

                uT_ps = ps0.tile([H, B], F32, tag="mm")
                nc.tensor.matmul(uT_ps[:], lhsT=wsb["cW2"][:], rhs=cu1T[:], start=True, stop=True)
                uT = p0.tile([H, B], F32, tag="uT")
                nc.scalar.activation(uT[:], uT_ps[:], mybir.ActivationFunctionType.Identity,
                                     bias=wsb["cb2c"][:, 0:1])

                ub_ps = ps0.tile([B, KW // 2], F32, tag="mm")
                nc.tensor.matmul(ub_ps[:], lhsT=uT[:], rhs=wsb["kW1u"][:], start=True, stop=False)
                nc.tensor.matmul(ub_ps[:], lhsT=ones_r[:, :B], rhs=wsb["kb1r"][:], start=False, stop=True)
                nc.scalar.activation(ubias[:], ub_ps[:], mybir.ActivationFunctionType.Copy)

                u_ps = ps0b.tile([B, H], F32, tag="tp")
                nc.tensor.transpose(u_ps[:], in_=uT[:], identity=ident[:H, :H])
                u_sb = p0.tile([B, H], F32, tag="u_sb")
                nc.vector.tensor_copy(u_sb[:], u_ps[:])
                nc.sync.dma_start(u_o[:], u_sb[:])

                # node encoder: full table (replicated) -> h0_dram
                xT_t = p0big.tile([NIN, npad], F32, tag="xT")
                nc.sync.dma_start(xT_t[:], xT_i[:])
                for gg in range(nblk_global):
                    sl = slice(gg * P, (gg + 1) * P)
                    t1_ps = ps0.tile([H, P], F32, tag="mm")
                    nc.tensor.matmul(t1_ps[:], lhsT=wsb["nW1"][:], rhs=xT_t[:, sl], start=True, stop=True)
                    t1 = p0.tile([H, P], F32, tag="t1")
                    nc.scalar.activation(t1[:], t1_ps[:], mybir.ActivationFunctionType.Relu,
                                         bias=wsb["nb1c"][:, 0:1])
                    h0_ps = ps0.tile([H, P], F32, tag="mm")
                    nc.tensor.matmul(h0_ps[:], lhsT=wsb["nW2"][:], rhs=t1[:], start=True, stop=True)
                    h0T_g = p0.tile([H, P], F32, tag="h0T")
                    nc.scalar.activation(h0T_g[:], h0_ps[:], mybir.ActivationFunctionType.Identity,
                                         bias=wsb["nb2c"][:, 0:1])
                    tp_ps = ps0b.tile([P, H], F32, tag="tp")
                    nc.tensor.transpose(tp_ps[:], in_=h0T_g[:], identity=ident[:H, :H])
                    h0n = p0.tile([P, H], F32, tag="h0n")
                    nc.vector.tensor_copy(h0n[:], tp_ps[:])
                    nc.sync.dma_start(h0_dram[sl, :], h0n[:])

                # own-slice transposed h0 -> hT_a
                xTo_t = p0big.tile([NIN, percore], F32, tag="xTo")
                nc.sync.dma_start(xTo_t[:], xTo_i[:])
                for g in range(ngroups):
                    sl = slice(g * P, (g + 1) * P)
                    t1_ps = ps0.tile([H, P], F32, tag="mm")
                    nc.tensor.matmul(t1_ps[:], lhsT=wsb["nW1"][:], rhs=xTo_t[:, sl], start=True, stop=True)
                    t1 = p0.tile([H, P], F32, tag="t1")
                    nc.scalar.activation(t1[:], t1_ps[:], mybir.ActivationFunctionType.Relu,
                                         bias=wsb["nb1c"][:, 0:1])
                    h0_ps = ps0.tile([H, P], F32, tag="mm")
                    nc.tensor.matmul(h0_ps[:], lhsT=wsb["nW2"][:], rhs=t1[:], start=True, stop=True)
                    nc.scalar.activation(hT_a[:, sl], h0_ps[:], mybir.ActivationFunctionType.Identity,
                                         bias=wsb["nb2c"][:, 0:1])

            # ---------- phase 1 (fused with layer 0) + layers ----------
            with tc.tile_pool(name="lay", bufs=1) as pl, \
                 tc.tile_pool(name="layw", bufs=3) as plw, \
                 tc.tile_pool(name="laytmp", bufs=2) as plt, \
                 tc.tile_pool(name="layps", bufs=2, space="PSUM") as psl, \
                 tc.tile_pool(name="laytp", bufs=2, space="PSUM") as pst:

                def gather_hsrc(table):
                    hsrc = pl.tile([P, T * H], F32, tag="hsrc", name="hsrc")
                    GCH = 8  # tiles per gather chunk (1024 descriptors)
                    for c0 in range(0, T, GCH):
                        tc_n = min(GCH, T - c0)
                        nc.gpsimd.dma_gather(
                            out_ap=hsrc[:, c0 * H:(c0 + tc_n) * H].rearrange(
                                "p (t e) -> p t e", t=tc_n),
                            in_ap=table[:, :],
                            idxs_ap=srci_t[:, c0 * 8:(c0 + tc_n) * 8],
                            num_idxs=tc_n * P,
                            num_idxs_reg=tc_n * P,
                            elem_size=H,
                        )
                    hsrc_bf = pl.tile([P, T * H], BF16, tag="hsrc_bf", name="hsrc_bf")
                    nc.vector.tensor_copy(hsrc_bf[:], hsrc[:])
                    return hsrc_bf

                def mix_scatter(t, w_ap, hsrc_bf, r_ps):
                    tmp = plt.tile([P, H * H], BF16, tag="tmp", name="tmp")
                    nc.vector.tensor_tensor(
                        out=tmp[:].rearrange("p (o i) -> p o i", o=H),
                        in0=w_ap.rearrange("p (o i) -> p o i", o=H),
                        in1=hsrc_bf[:, t * H:(t + 1) * H][:, None, :].to_broadcast([P, H, H]),
                        op=mybir.AluOpType.mult,
                    )
                    msg_t = plt.tile([P, H], F32, tag="msg_t", name="msg_t")
                    nc.vector.tensor_reduce(
                        out=msg_t[:], in_=tmp[:].rearrange("p (o i) -> p o i", o=H),
                        axis=mybir.AxisListType.X, op=mybir.AluOpType.add)
                    s_t = plt.tile([P, P], F32, tag="s_t", name="s_t")
                    nc.vector.tensor_scalar(
                        out=s_t[:], in0=iota_f[:],
                        scalar1=dstr_t[:, t:t + 1], scalar2=rcpe_t[:, t:t + 1],
                        op0=mybir.AluOpType.is_equal, op1=mybir.AluOpType.mult)
                    nc.tensor.matmul(r_ps[:], lhsT=s_t[:], rhs=msg_t[:],
                                     start=False, stop=False)

                def finalize_group(l, g, r_ps, hT_next):
                    nc.tensor.matmul(r_ps[:], lhsT=ones_r[:], rhs=wsb["cbias_r"][:],
                                     start=False, stop=True)
                    h_new = plt.tile([P, H], F32, tag="h_new", name="h_new")
                    nc.scalar.activation(h_new[:], r_ps[:], mybir.ActivationFunctionType.Relu)
                    tp_ps = pst.tile([H, P], F32, tag="tp", name="tp_ps")
                    nc.tensor.transpose(tp_ps[:], in_=h_new[:], identity=ident[:])
                    gsl = slice(g * P, (g + 1) * P)
                    if l < LAYERS - 1:
                        nc.sync.dma_start(cc_in[l][gsl, :], h_new[:])
                        nc.vector.tensor_copy(hT_next[:, gsl], tp_ps[:])
                    else:
                        h3T_g = plt.tile([H, P], F32, tag="h3T", name="h3T_g")
                        nc.vector.tensor_copy(h3T_g[:], tp_ps[:])
                        o_ps = psl.tile([P, OUT], F32, tag="g", name="o_ps")
                        nc.tensor.matmul(o_ps[:], lhsT=h3T_g[:], rhs=wsb["oW"][:],
                                         start=True, stop=False)
                        nc.tensor.matmul(o_ps[:], lhsT=ones_r[:], rhs=wsb["ob_r"][:],
                                         start=False, stop=True)
                        nc.scalar.activation(out_sb[:, g:g + 1], o_ps[:],
                                             mybir.ActivationFunctionType.Copy)

                # fused: kernel-network w production + layer-0 message passing
                with tc.tile_pool(name="ph1big", bufs=1) as p1big, \
                     tc.tile_pool(name="ph1", bufs=2) as p1, \
                     tc.tile_pool(name="ph1w", bufs=2) as p1w, \
                     tc.tile_pool(name="ph1ps", bufs=2, space="PSUM") as ps1, \
                     tc.tile_pool(name="ph1psw", bufs=2, space="PSUM") as ps1w:

                    eaT_t = p1big.tile([EIN, emax], F32, tag="eaT")
                    nc.sync.dma_start(eaT_t[:], eaT_i[:])
                    onebf = p1big.tile([B, emax], F32, tag="onebf")
                    nc.sync.dma_start(onebf[:], oneb_i[:])

                    hsrc_bf = gather_hsrc(h0_dram)
                    tile_idx = 0
                    for g in range(ngroups):
                        r_ps = psl.tile([P, H], F32, tag="g", name="r_ps")
                        nc.tensor.matmul(r_ps[:], lhsT=hT_a[:, g * P:(g + 1) * P],
                                         rhs=wsb["root"][:], start=True, stop=False)
                        for _s in range(subt[g]):
                            t = tile_idx
                            tile_idx += 1
                            sl = slice(t * P, (t + 1) * P)
                            k1_ps = ps1.tile([KW // 2, P], F32, tag="k", name="k1_ps")
                            nc.tensor.matmul(k1_ps[:], lhsT=wsb["kW1a"][:], rhs=eaT_t[:, sl],
                                             start=True, stop=False)
                            nc.tensor.matmul(k1_ps[:], lhsT=ubias[:], rhs=onebf[:, sl],
                                             start=False, stop=True)
                            k1T = p1.tile([KW // 2, P], BF16, tag="k1T", name="k1T")
                            nc.scalar.activation(k1T[:], k1_ps[:], mybir.ActivationFunctionType.Relu)

                            k2T = []
                            for half in range(2):
                                k2_ps = ps1.tile([KW // 2, P], F32, tag="k", name="k2_ps")
                                nc.tensor.matmul(k2_ps[:], lhsT=wsb["kW2bf"][:, half * 128:(half + 1) * 128],
                                                 rhs=k1T[:], start=True, stop=True)
                                k2h = p1.tile([KW // 2, P], BF16, tag=f"k2T{half}", name="k2h")
                                nc.scalar.activation(k2h[:], k2_ps[:], mybir.ActivationFunctionType.Relu,
                                                     bias=wsb["kb2c"][:, half:half + 1])
                                k2T.append(k2h)

                            w_sb = p1w.tile([P, H * H], BF16, tag="w_sb", name="w_sb")
                            for chunk in range(NCH):
                                csl = slice(chunk * WCHUNK, (chunk + 1) * WCHUNK)
                                w_ps = ps1w.tile([P, WCHUNK], F32, tag="w", name="w_ps")
                                nc.tensor.matmul(w_ps[:], lhsT=k2T[0][:], rhs=wsb["kW3p0"][:, csl],
                                                 start=True, stop=False)
                                nc.tensor.matmul(w_ps[:], lhsT=k2T[1][:], rhs=wsb["kW3p1"][:, csl],
                                                 start=False, stop=False)
                                nc.tensor.matmul(w_ps[:], lhsT=ones_bf[:], rhs=wsb["kb3p"][:, csl],
                                                 start=False, stop=True)
                                nc.scalar.activation(w_sb[:, csl], w_ps[:], mybir.ActivationFunctionType.Copy)
                            nc.sync.dma_start(w_dram[sl, :], w_sb[:])
                            mix_scatter(t, w_sb[:], hsrc_bf, r_ps)
                        finalize_group(0, g, r_ps, hT_b)

                nc.gpsimd.collective_compute(
                    "AllGather", mybir.AluOpType.bypass,
                    replica_groups=[list(range(NC))],
                    ins=[cc_in[0][:].opt()], outs=[cc_out[0][:].opt()])

                hT_cur, hT_next = hT_b, hT_a
                for l in range(1, LAYERS):
                    hsrc_bf = gather_hsrc(cc_out[l - 1])
                    tile_idx = 0
                    for g in range(ngroups):
                        r_ps = psl.tile([P, H], F32, tag="g", name="r_ps")
                        nc.tensor.matmul(r_ps[:], lhsT=hT_cur[:, g * P:(g + 1) * P],
                                         rhs=wsb["root"][:], start=True, stop=False)
                        for _s in range(subt[g]):
                            t = tile_idx
                            tile_idx += 1
                            w_t = plw.tile([P, H * H], BF16, tag="w_t", name="w_t")
                            nc.sync.dma_start(w_t[:], w_dram[t * P:(t + 1) * P, :])
                            mix_scatter(t, w_t[:], hsrc_bf, r_ps)
                        finalize_group(l, g, r_ps, hT_next)

                    if l < LAYERS - 1:
                        nc.gpsimd.collective_compute(
                            "AllGather", mybir.AluOpType.bypass,
                            replica_groups=[list(range(NC))],
                            ins=[cc_in[l][:].opt()], outs=[cc_out[l][:].opt()])
                        hT_cur, hT_next = hT_next, hT_cur

                nc.sync.dma_start(
                    out_o[:].rearrange("(g p) one -> p g one", p=P),
                    out_sb[:, :, None])

    nc.compile()
    return nc


def run_device(inputs, cfg=None, trace=False):
    """Build + execute on the 8 NeuronCores; needs jax to see the axon devices."""
    from concourse import bass_utils

    cfg = cfg or CFG
    in_maps, meta = host_prep(inputs, cfg)
    nc = build_program(cfg, meta)
    res = bass_utils.run_bass_kernel_spmd(
        nc, in_maps, core_ids=list(range(cfg["NCORES"])), trace=trace)
    out = np.concatenate([res.results[c]["out_o"] for c in range(cfg["NCORES"])], axis=0)
    out = out[: cfg["N"]]
    u = res.results[0]["u_o"]
    return (out.astype(np.float32), u.astype(np.float32)), res


def _neuron_devices_visible():
    try:
        import jax

        return len(jax.devices()) >= CFG["NCORES"]
    except Exception:
        return False


def kernel(**inputs):
    if _neuron_devices_visible():
        (out, u), _ = run_device(inputs)
        return out, u

    # jax in this process cannot see the NeuronCores (e.g. JAX_PLATFORMS=cpu
    # was set for the host-side reference) — re-exec in a clean subprocess.
    with tempfile.TemporaryDirectory() as td:
        np.savez(os.path.join(td, "in.npz"),
                 **{k: np.asarray(v) for k, v in inputs.items()})
        env = dict(os.environ)
        for k in ["JAX_PLATFORMS", "JAX_PLATFORM_NAME"]:
            if "cpu" in env.get(k, "").lower():
                env.pop(k)
        subprocess.run(
            [sys.executable, os.path.abspath(__file__), "--runner", td],
            check=True, env=env,
        )
        with np.load(os.path.join(td, "out.npz")) as z:
            return z["out"], z["u"]


def _runner_main(td):
    with np.load(os.path.join(td, "in.npz")) as z:
        inputs = {k: z[k] for k in z.files}
    (out, u), _ = run_device(inputs)
    np.savez(os.path.join(td, "out.npz"), out=out, u=u)


if __name__ == "__main__" and len(sys.argv) == 3 and sys.argv[1] == "--runner":
    _runner_main(sys.argv[2])
